# revision 13
# baseline (speedup 1.0000x reference)
"""Trainium2 Bass kernel: CrossAttention (B=2, Nq=1024, Nkv=2048, D=1024, H=16).

Sharding: 8 cores = 2 (batch) x 4 (head groups of 4 heads).
Each core computes, for its batch b and heads [4g, 4g+4):
    qT = (x_b @ Wq_s)^T            [256, 1024]   (dd on partitions)
    kT = (mem_b @ Wk_s)^T          [256, 2048]
    v  = mem_b @ Wv_s              [2048, 256]   (+ ones column per head)
    per head: sT = k_h @ q_h^T     [2048, 1024]  (j on partitions)
              eT = exp(SCALE*sT)
              cu = [v_h | 1]^T-acc [65, 1024]    (row 64 = softmax denom)
              ctx_h = cu[0:64] * recip(cu[64]) broadcast
    part = ctx @ Wp_s              [1024, 1024]  (row-parallel partial)
Host sums the 4 partials per batch and adds b_proj.

v3 scheduling notes (engine streams follow emission order, so interleave):
 - All matmul operands bf16; PSUM f32; exp on ACT is the attention-phase
   floor (~73us of ACTIVATE) while PE carries ~105us total -> PE must
   never idle.  DMA order: wq, xt*8 (qT streams from ~3us), wk, memt
   kv-half-0 in 4 kc-pair chunks (kT(0,0) lands just before qT ends),
   then wv / kv-half-1 / wp.
 - h0's jc loop interleaves v_chunk(jc) between QK and PV; kT quarters
   and the first output-projection half run as PE filler inside later
   heads' loops.
 - Softmax denominator per head: DVE copy of den -> ones-matmul broadcast
   -> DVE reciprocal_approx_fast -> DVE mul, emitted one head late with
   PE filler in between so the PE never sits on the chain.
 - Output projection split into two accumulation groups: h0+h1 partials
   (filler inside h2/h3, copied to SBUF) and h2+h3 at the end, combined
   with a DVE add.  Halves the serial projection tail.
 - ACT only ever runs Exp (+Copy): one table load, warmed at t=0.
"""

import numpy as np
import ml_dtypes

DIM = 1024
HEADS = 16
HD = 64
B = 2
NQ = 1024
NKV = 2048
SCALE = HD ** -0.5
N_CORES = 8
HG = 4               # heads per core
DD = HG * HD         # 256 packed head dims per core
KC = 8               # contraction chunks (DIM / 128)
JC = NKV // 128      # 16 kv-row chunks

_CACHE = {}


def _build_module():
    import concourse.bacc as bacc
    import concourse.tile as tile
    import concourse.mybir as mybir

    f32 = mybir.dt.float32
    f32r = mybir.dt.float32r
    bf16 = mybir.dt.bfloat16
    EXP = mybir.ActivationFunctionType.Exp

    nc = bacc.Bacc(
        trn_type="TRN2",
        target_bir_lowering=False,
        debug=False,
        num_devices=N_CORES,
    )

    xt_d = nc.dram_tensor("xt", [128, KC, NQ], bf16, kind="ExternalInput").ap()
    memt_d = nc.dram_tensor(
        "memt", [128, KC, 2, NKV // 2], bf16, kind="ExternalInput"
    ).ap()
    wq_d = nc.dram_tensor("wq", [128, KC, DD], bf16, kind="ExternalInput").ap()
    wk_d = nc.dram_tensor("wk", [128, KC, DD], bf16, kind="ExternalInput").ap()
    wv_d = nc.dram_tensor("wv", [128, KC, DD], bf16, kind="ExternalInput").ap()
    wp_d = nc.dram_tensor("wp", [64, HG, DIM], bf16, kind="ExternalInput").ap()
    ones_d = nc.dram_tensor("ones_in", [1, HD], f32r, kind="ExternalInput").ap()
    vones_d = nc.dram_tensor(
        "vones", [128, JC * HG], f32r, kind="ExternalInput"
    ).ap()
    out_d = nc.dram_tensor("out", [NQ, DIM], f32, kind="ExternalOutput").ap()

    with tile.TileContext(nc) as tc:
        with (
            tc.tile_pool(name="wpool", bufs=1) as wpool,
            tc.tile_pool(name="persist", bufs=1) as persist,
            tc.tile_pool(name="xstream", bufs=3) as xstream,
            tc.tile_pool(name="work", bufs=2) as work,
            tc.tile_pool(name="opool", bufs=2) as opool,
            tc.tile_pool(name="psum", bufs=3, space="PSUM") as psum,
        ):
            # ---- tiny inputs first + ACT exp-table warm-up at t=0 ----
            ones_sb = wpool.tile([65, HD], f32r, name="ones_sb")
            nc.sync.dma_start(out=ones_sb[64:65, :], in_=ones_d)
            vones_sb = wpool.tile([128, JC * HG], f32r, name="vones_sb")
            nc.sync.dma_start(out=vones_sb, in_=vones_d)
            warm_sb = wpool.tile([1, 16], f32, name="warm_sb")
            nc.scalar.activation(out=warm_sb, in_=ones_sb[64:65, 0:16], func=EXP)

            wq_sb = wpool.tile([128, KC, DD], bf16, name="wq_sb")
            nc.sync.dma_start(out=wq_sb, in_=wq_d)

            wk_sb = wpool.tile([128, KC, DD], bf16, name="wk_sb")
            memt_sb = [[None] * KC, [None] * KC]

            # ---- qT projection, xt streaming chunk by chunk; wk + kv-half-0
            # DMA chunks interleaved so kT(0,0) can start as qT drains ----
            qT_sb = persist.tile([128, 2, NQ], bf16, name="qT_sb")
            qt_ps = [
                psum.tile([128, NQ], f32, name=f"qt_ps{mc}", tag="ps", bufs=3)
                for mc in range(2)
            ]
            for kc in range(KC):
                xt_sb = xstream.tile([128, NQ], bf16, name="xt_sb", tag="xt", bufs=3)
                nc.sync.dma_start(out=xt_sb, in_=xt_d[:, kc, :])
                for mc in range(2):
                    for ih in range(2):
                        nc.tensor.matmul(
                            qt_ps[mc][:, ih * 512 : (ih + 1) * 512],
                            lhsT=wq_sb[:, kc, mc * 128 : (mc + 1) * 128],
                            rhs=xt_sb[:, ih * 512 : (ih + 1) * 512],
                            start=(kc == 0),
                            stop=(kc == KC - 1),
                        )
                if kc == 1:
                    nc.sync.dma_start(out=wk_sb, in_=wk_d)
                if 2 <= kc <= 5:
                    i = kc - 2
                    m = wpool.tile(
                        [128, 2, NKV // 2], bf16, name=f"memt0_{i}", tag=f"memt0{i}"
                    )
                    nc.sync.dma_start(out=m, in_=memt_d[:, 2 * i : 2 * i + 2, 0, :])
                    memt_sb[0][2 * i] = m[:, 0, :]
                    memt_sb[0][2 * i + 1] = m[:, 1, :]
            for mc in range(2):
                nc.vector.tensor_copy(out=qT_sb[:, mc, :], in_=qt_ps[mc])

            wv_sb = wpool.tile([128, KC, DD], bf16, name="wv_sb")
            nc.sync.dma_start(out=wv_sb, in_=wv_d)
            memt1_sb = wpool.tile([128, KC, NKV // 2], bf16, name="memt1_sb")
            nc.sync.dma_start(out=memt1_sb, in_=memt_d[:, :, 1, :])
            for kc in range(KC):
                memt_sb[1][kc] = memt1_sb[:, kc, :]
            wp_sb = wpool.tile([64, HG, DIM], bf16, name="wp_sb")
            nc.sync.dma_start(out=wp_sb, in_=wp_d)

            # ---- persistent intermediates ----
            kT_sb = persist.tile([128, 2, NKV], bf16, name="kT_sb")
            vaug_sb = persist.tile([128, JC, HG, HD + 1], bf16, name="vaug_sb")
            ctx_sb = persist.tile([64, HG, NQ], bf16, name="ctx_sb")
            outA_sb = persist.tile([128, 8, DIM], f32, name="outA_sb")

            nc.vector.tensor_copy(
                out=vaug_sb[:, :, :, HD : HD + 1],
                in_=vones_sb.rearrange("p (j h) -> p j h", j=JC)[:, :, :, None],
            )

            # ---- building blocks ----
            def kt_quarter(mc, jh2):
                kt_ps = psum.tile(
                    [128, NKV // 2], f32, name=f"kt_ps_{mc}_{jh2}", tag="ps", bufs=3
                )
                for kc in range(KC):
                    for jh in range(2):
                        nc.tensor.matmul(
                            kt_ps[:, jh * 512 : (jh + 1) * 512],
                            lhsT=wk_sb[:, kc, mc * 128 : (mc + 1) * 128],
                            rhs=memt_sb[jh2][kc][:, jh * 512 : (jh + 1) * 512],
                            start=(kc == 0),
                            stop=(kc == KC - 1),
                        )
                nc.vector.tensor_copy(
                    out=kT_sb[:, mc, jh2 * 1024 : (jh2 + 1) * 1024], in_=kt_ps
                )

            def v_chunk(jc):
                jh2, jr = divmod(jc, 8)
                v_ps = psum.tile([128, DD], f32, name=f"v_ps{jc}", tag="ps", bufs=3)
                for kc in range(KC):
                    nc.tensor.matmul(
                        v_ps,
                        lhsT=memt_sb[jh2][kc][:, jr * 128 : (jr + 1) * 128],
                        rhs=wv_sb[:, kc, :],
                        start=(kc == 0),
                        stop=(kc == KC - 1),
                    )
                nc.vector.tensor_copy(
                    out=vaug_sb[:, jc, :, 0:HD],
                    in_=v_ps.rearrange("p (h d) -> p h d", h=HG),
                )

            cu = [None] * HG

            def qk_exp(h, jc):
                hp = h // 2
                po = (h % 2) * 64
                sT = psum.tile(
                    [128, NQ], f32, name=f"sT_ps_{h}_{jc}", tag="ps", bufs=3
                )
                for ih in range(2):
                    nc.tensor.matmul(
                        sT[:, ih * 512 : (ih + 1) * 512],
                        lhsT=kT_sb[po : po + 64, hp, jc * 128 : (jc + 1) * 128],
                        rhs=qT_sb[po : po + 64, hp, ih * 512 : (ih + 1) * 512],
                        start=True,
                        stop=True,
                    )
                eT = work.tile([128, NQ], bf16, name="eT_sb", tag="eT", bufs=8)
                nc.scalar.activation(out=eT, in_=sT, func=EXP, scale=SCALE)
                return eT

            def pv(h, jc, eT):
                for ih in range(2):
                    nc.tensor.matmul(
                        cu[h][:, ih * 512 : (ih + 1) * 512],
                        lhsT=vaug_sb[:, jc, h, :],
                        rhs=eT[:, ih * 512 : (ih + 1) * 512],
                        start=(jc == 0),
                        stop=(jc == JC - 1),
                    )

            def denom_pre(h):
                # den copy + broadcast matmul (PE part of the chain)
                den_sb = work.tile([65, NQ], f32r, name="den_sb", tag="den", bufs=2)
                nc.vector.tensor_copy(out=den_sb[64:65, :], in_=cu[h][64:65, :])
                bden_ps = psum.tile(
                    [64, NQ], f32, name=f"bden_ps{h}", tag="ps", bufs=3
                )
                for ih in range(2):
                    nc.tensor.matmul(
                        bden_ps[:, ih * 512 : (ih + 1) * 512],
                        lhsT=ones_sb[64:65, :],
                        rhs=den_sb[64:65, ih * 512 : (ih + 1) * 512],
                        start=True,
                        stop=True,
                    )
                return bden_ps

            def denom_post(h, bden_ps):
                inv_sb = work.tile([64, NQ], f32, name="inv_sb", tag="inv", bufs=2)
                nc.vector.reciprocal_approx_fast(out=inv_sb, in_=bden_ps)
                nc.vector.tensor_mul(ctx_sb[:, h, :], cu[h][0:HD, :], inv_sb)

            def proj_a(ic):
                # first projection half: heads 0+1, parked in SBUF f32
                prA_ps = psum.tile(
                    [128, DIM], f32, name=f"prA_ps{ic}", tag="ps", bufs=3
                )
                for hh in range(2):
                    for nh in range(2):
                        nc.tensor.matmul(
                            prA_ps[:, nh * 512 : (nh + 1) * 512],
                            lhsT=ctx_sb[:, hh, ic * 128 : (ic + 1) * 128],
                            rhs=wp_sb[:, hh, nh * 512 : (nh + 1) * 512],
                            start=(hh == 0),
                            stop=(hh == 1),
                        )
                nc.vector.tensor_copy(out=outA_sb[:, ic, :], in_=prA_ps)

            def proj_b(ic):
                prB_ps = psum.tile(
                    [128, DIM], f32, name=f"prB_ps{ic}", tag="ps", bufs=3
                )
                for hh in range(2, 4):
                    for nh in range(2):
                        nc.tensor.matmul(
                            prB_ps[:, nh * 512 : (nh + 1) * 512],
                            lhsT=ctx_sb[:, hh, ic * 128 : (ic + 1) * 128],
                            rhs=wp_sb[:, hh, nh * 512 : (nh + 1) * 512],
                            start=(hh == 2),
                            stop=(hh == 3),
                        )
                out_sb = opool.tile([128, DIM], f32, name="out_sb", tag="out")
                nc.vector.tensor_add(out_sb, prB_ps, outA_sb[:, ic, :])
                nc.sync.dma_start(
                    out=out_d[ic * 128 : (ic + 1) * 128, :], in_=out_sb
                )

            # ---- attention: heads sequential, PE filler interleaved.
            # PV lags QK by one jc so it never waits on the current exp;
            # kT sub-quarters / projection chunks are sprinkled as 2-4 mm
            # granules to keep the PE dense (HAM stays warm) while ACT paces.
            def attn_head(h, filler):
                cu[h] = psum.tile(
                    [HD + 1, NQ], f32, name=f"cu_ps{h}", tag="cu", bufs=1
                )
                pend = []
                for jc in range(JC):
                    eT = qk_exp(h, jc)
                    f = filler.get(jc)
                    if f:
                        for fn in f:
                            fn()
                    pend.append((jc, eT))
                    if jc >= 1:
                        j2, e2 = pend.pop(0)
                        pv(h, j2, e2)
                for j2, e2 in pend:
                    pv(h, j2, e2)

            def kt_sub(mc, jh2, ki):
                # quarter split into kc-pair granules (4 mms each)
                kt_ps = kt_state.get((mc, jh2))
                if kt_ps is None:
                    kt_ps = psum.tile(
                        [128, NKV // 2], f32, name=f"kt_ps_{mc}_{jh2}",
                        tag="ps", bufs=3,
                    )
                    kt_state[(mc, jh2)] = kt_ps
                for kc in (2 * ki, 2 * ki + 1):
                    for jh in range(2):
                        nc.tensor.matmul(
                            kt_ps[:, jh * 512 : (jh + 1) * 512],
                            lhsT=wk_sb[:, kc, mc * 128 : (mc + 1) * 128],
                            rhs=memt_sb[jh2][kc][:, jh * 512 : (jh + 1) * 512],
                            start=(kc == 0),
                            stop=(kc == KC - 1),
                        )
                if ki == 3:
                    nc.vector.tensor_copy(
                        out=kT_sb[:, mc, jh2 * 1024 : (jh2 + 1) * 1024],
                        in_=kt_ps,
                    )
                    del kt_state[(mc, jh2)]

            kt_state = {}

            # h0: v chunks are the natural filler
            cu[0] = psum.tile([HD + 1, NQ], f32, name="cu_ps0", tag="cu", bufs=1)
            kt_quarter(0, 0)
            pend = []
            for jc in range(JC):
                if jc == 6:
                    kt_quarter(0, 1)
                eT = qk_exp(0, jc)
                v_chunk(jc)
                pend.append((jc, eT))
                if jc >= 1:
                    j2, e2 = pend.pop(0)
                    pv(0, j2, e2)
            for j2, e2 in pend:
                pv(0, j2, e2)

            # h1: delayed h0 denominator, kT(1,*) spread in 4-mm granules
            bd = {}
            attn_head(1, {
                0: [lambda: bd.__setitem__(0, denom_pre(0)),
                    lambda: kt_sub(1, 0, 0),
                    lambda: denom_post(0, bd[0])],
                2: [lambda: kt_sub(1, 0, 1)],
                4: [lambda: kt_sub(1, 0, 2)],
                6: [lambda: kt_sub(1, 0, 3)],
                8: [lambda: kt_sub(1, 1, 0)],
                10: [lambda: kt_sub(1, 1, 1)],
                12: [lambda: kt_sub(1, 1, 2)],
                14: [lambda: kt_sub(1, 1, 3)],
            })

            # h2: delayed h1 denominator + projection(h0+h1) chunks
            attn_head(2, {
                0: [lambda: bd.__setitem__(1, denom_pre(1)),
                    lambda: denom_post(1, bd[1])],
                1: [lambda: proj_a(0)],
                3: [lambda: proj_a(1)],
                5: [lambda: proj_a(2)],
                7: [lambda: proj_a(3)],
                9: [lambda: proj_a(4)],
                11: [lambda: proj_a(5)],
            })

            # h3: delayed h2 denominator + last projection-a chunks
            attn_head(3, {
                0: [lambda: bd.__setitem__(2, denom_pre(2)),
                    lambda: denom_post(2, bd[2])],
                1: [lambda: proj_a(6)],
                3: [lambda: proj_a(7)],
            })

            bd3 = denom_pre(3)
            denom_post(3, bd3)

            # ---- second projection half + combine + store ----
            for ic in range(8):
                proj_b(ic)

    nc.compile()
    return nc


def get_module():
    if "nc" not in _CACHE:
        _CACHE["nc"] = _build_module()
    return _CACHE["nc"]


def make_in_maps(x, mem, W_kv, W_q, W_proj):
    """Host-side shard + repack into the k-major bf16 layouts."""
    bf = ml_dtypes.bfloat16
    x = np.ascontiguousarray(np.asarray(x, np.float32))
    mem = np.ascontiguousarray(np.asarray(mem, np.float32))
    W_kv = np.asarray(W_kv, np.float32)
    W_q = np.asarray(W_q, np.float32)
    W_proj = np.asarray(W_proj, np.float32)

    def pack_k(a):  # [1024, N] -> [128, 8, N] bf16, k-chunked
        n = a.shape[1]
        return (
            np.ascontiguousarray(a.reshape(KC, 128, n).transpose(1, 0, 2))
            .astype(bf)
        )

    xt_b = [pack_k(x[b].T) for b in range(B)]
    memt_b = [
        np.ascontiguousarray(
            pack_k(mem[b].T).reshape(128, KC, 2, NKV // 2)
        )
        for b in range(B)
    ]
    ones = np.ones((1, HD), np.float32)
    vones = np.ones((128, JC * HG), np.float32)

    in_maps = []
    for core in range(N_CORES):
        b, g = divmod(core, 4)
        cs = slice(g * DD, (g + 1) * DD)
        wq = pack_k(W_q[:, cs])
        wk = pack_k(W_kv[:, :DIM][:, cs])
        wv = pack_k(W_kv[:, DIM:][:, cs])
        # wp[d, h, n] = W_proj[g*256 + h*64 + d, n]
        wp = (
            np.ascontiguousarray(
                W_proj[cs, :].reshape(HG, HD, DIM).transpose(1, 0, 2)
            )
            .astype(bf)
        )
        in_maps.append(
            {
                "xt": xt_b[b],
                "memt": memt_b[b],
                "wq": wq,
                "wk": wk,
                "wv": wv,
                "wp": wp,
                "ones_in": ones,
                "vones": vones,
            }
        )
    return in_maps


def combine_outputs(partials, b_proj):
    """Sum the 4 row-parallel partials per batch, add bias."""
    b_proj = np.asarray(b_proj, np.float32)
    out = np.zeros((B, NQ, DIM), np.float32)
    for core in range(N_CORES):
        out[core // 4] += np.asarray(partials[core], np.float32)
    out += b_proj[None, None, :]
    return out


def kernel(x, mem, W_kv, W_q, W_proj, b_proj):
    from concourse import bass_utils

    nc = get_module()
    in_maps = make_in_maps(x, mem, W_kv, W_q, W_proj)
    res = bass_utils.run_bass_kernel_spmd(
        nc, in_maps, core_ids=list(range(N_CORES))
    )
    partials = [res.results[c]["out"] for c in range(N_CORES)]
    return combine_outputs(partials, b_proj)
